# revision 9
# baseline (speedup 1.0000x reference)
"""Trainium2 Bass kernel for the CRF problem (v2).

Math:
  feat = conv2d(X.view(-1,1,16,8), K, pad=2)  -> flatten      (B, L, D)
  e    = feat @ W                                              (B, L, Y)
Both are linear in X, so fold:  e = X @ G  with  G = C_K @ W  (D x Y),
C_K the 128x128 conv matrix built from the 5x5 kernel (host prep, tiny).

logZ via the *scaled* forward algorithm, with fwd and bwd chains fused
into ONE state tile so every chain matmul uses the same stationary
block-diagonal weights (no per-link weight swaps):

  S tile [128, 256]: partitions = 4 strips of 32 states.
    strips 0,1 = fwd alpha for words 0..255 / 256..511 (cols)
    strips 2,3 = bwd gamma for the same words
  BDFX = diag(M, M, M^T, M^T)  with M = exp(T)/Y.
  link k (1..31):  S_k = E_k ⊙ (BDFX-blockdiag @ S_{k-1})
    where E_k[strips 0,1] = exp(e) at t=k, E_k[strips 2,3] = exp(e) at
    t=63-k. Init S_0 = E_0 (alpha_0 = E_0 top, gamma_63 = E_63 bottom).
  final: beta_31 = M^T gamma_32 moved to partitions 0..63 via a
    shifted weight block; u = alpha_31 ⊙ beta_31; logZ = ln Σ_y u
    + 63 ln Y per word.

Inputs stream in fp8 (e3m4): XT [128, 32768] and the em one-hot OHT.
G (x256 for fp8 range) multiplies on the PE in 4 column strips.
em score: ACT copies raw e to SBUF (scaled back), GPSIMD does the
masked accumulate — DVE stays dedicated to the latency-critical chain.
"""

import numpy as np
import ml_dtypes

B, L, D, Y = 4096, 64, 128, 26
NCORES = 8
WPC = B // NCORES          # 512 words per core
NT = 8                     # tau blocks; tau covers links k=4t..4t+3
C_REG = 1000.0
GS = 256.0                 # G scale so fp8e3 stays in normal range
NWARM = 14                 # PE warm-up matmuls (keep HAM at K=8/8)

_BF16 = ml_dtypes.bfloat16
_F8 = ml_dtypes.float8_e3m4
_PROG = {}


def _conv_matrix(K5):
    """C[q, p]: flattened-input q contribution to flattened-output p."""
    H, Wd = 16, 8
    C = np.zeros((D, D), dtype=np.float64)
    for oh in range(H):
        for ow in range(Wd):
            p = oh * Wd + ow
            for kh in range(5):
                for kw in range(5):
                    ih, iw = oh + kh - 2, ow + kw - 2
                    if 0 <= ih < H and 0 <= iw < Wd:
                        C[ih * Wd + iw, p] = K5[kh, kw]
    return C


def _build_program(reps=1):
    if reps in _PROG:
        return _PROG[reps]
    import concourse.tile as tile
    import concourse.mybir as mybir
    from concourse import bacc
    from concourse.bass import ds, ts

    f32 = mybir.dt.float32
    bf16 = mybir.dt.bfloat16
    f8 = mybir.dt.float8e3

    nc = bacc.Bacc("TRN2", target_bir_lowering=False, debug=False,
                   num_devices=NCORES)

    XT_d = nc.dram_tensor("XT", [D, WPC * L], f8, kind="ExternalInput")
    OHT_d = nc.dram_tensor("OHT", [128, NT * 1024], bf16,
                           kind="ExternalInput")
    G32_d = nc.dram_tensor("G32", [D, 32], f8, kind="ExternalInput")
    BDFX_d = nc.dram_tensor("BDFX", [128, 128], bf16, kind="ExternalInput")
    WFIN_d = nc.dram_tensor("WFIN", [128, 64], bf16, kind="ExternalInput")
    ONES_d = nc.dram_tensor("ONES2", [64, 2], bf16, kind="ExternalInput")
    EMP_d = nc.dram_tensor("EMP", [128, NT], f32, kind="ExternalOutput")
    LNS_d = nc.dram_tensor("LNS", [2, 256], f32, kind="ExternalOutput")

    with tile.TileContext(nc) as tc:
        with (
            tc.tile_pool(name="const", bufs=1) as cpool,
            tc.tile_pool(name="xt", bufs=3) as xtp,
            tc.tile_pool(name="ep", bufs=NT) as epool,
            tc.tile_pool(name="er", bufs=2) as erp,
            tc.tile_pool(name="scr", bufs=2) as scrp,
            tc.tile_pool(name="s", bufs=3) as spool,
            tc.tile_pool(name="out", bufs=1) as opool,
            tc.tile_pool(name="pe", bufs=2, space="PSUM") as pep,
            tc.tile_pool(name="pr", bufs=2, space="PSUM") as prp,
            tc.tile_pool(name="pf", bufs=1, space="PSUM") as pfp,
            tc.tile_pool(name="pl", bufs=1, space="PSUM") as plp,
        ):
            consts = {}
            emp = opool.tile([128, NT], f32)
            lns = opool.tile([2, 256], f32)

            # ---- PE warm-up: keep HAM busy while the first DMA lands ----
            warm = cpool.tile([128, 256], bf16)
            nc.vector.memset(warm[:], 0.0)
            pw = prp.tile([128, 256], f32, name="pw", tag="pr_t")
            for _ in range(NWARM):
                nc.tensor.matmul(pw[:], warm[:, 0:128], warm[:],
                                 start=True, stop=True)

            def load_consts():
                # via the gpsimd (SWDGE) queue: keeps the SP HWDGE ring
                # free for the XT stream
                g32 = cpool.tile([D, 32], f8)
                nc.gpsimd.dma_start(g32[:], G32_d[:])
                bdfx = cpool.tile([128, 128], bf16)
                nc.gpsimd.dma_start(bdfx[:], BDFX_d[:])
                wfin = cpool.tile([128, 64], bf16)
                nc.gpsimd.dma_start(wfin[:], WFIN_d[:])
                ones2 = cpool.tile([64, 2], bf16)
                nc.gpsimd.dma_start(ones2[:], ONES_d[:])
                oht = cpool.tile([128, NT * 1024], bf16)
                nc.gpsimd.dma_start(oht[:], OHT_d[:])
                consts.update(g32=g32, bdfx=bdfx, wfin=wfin, ones2=ones2,
                              oht=oht)

            for _rep in range(reps):
                xt_tiles = {}
                e_tiles = {}

                def produce_dma(t, split=1):
                    xt = xtp.tile([D, 4096], f8, name="xt_t", tag="xt_t")
                    if split == 1:
                        nc.sync.dma_start(xt[:], XT_d[:, ts(t, 4096)])
                    else:
                        for c in range(split):
                            w = 4096 // split
                            nc.sync.dma_start(
                                xt[:, ds(c * w, w)],
                                XT_d[:, ds(t * 4096 + c * w, w)])
                    xt_tiles[t] = xt

                def produce_tau(t):
                    xt = xt_tiles.pop(t)
                    pe = pep.tile([128, 1024], f32, name="pe_t", tag="pe_t")
                    for h in range(2):
                        for g in range(4):
                            nc.tensor.matmul(
                                pe[32 * g:32 * g + 32, ds(h * 512, 512)],
                                consts["g32"][:],
                                xt[:, ds(h * 2048 + g * 512, 512)],
                                start=True, stop=True,
                                tile_position=(0, 32 * g),
                            )
                    ep = epool.tile([128, 1024], bf16, name="e_t", tag="e_t")
                    nc.scalar.activation(ep[:], pe[:],
                                         mybir.ActivationFunctionType.Exp,
                                         scale=1.0 / GS)
                    er = erp.tile([128, 1024], bf16, name="er_t", tag="er_t")
                    nc.scalar.mul(er[:], pe[:], 1.0 / GS)
                    scr = scrp.tile([128, 1024], bf16, name="sc_t", tag="sc_t")
                    nc.vector.scalar_tensor_tensor(
                        out=scr[:], in0=er[:], scalar=1.0,
                        in1=consts["oht"][:, ts(t, 1024)],
                        op0=mybir.AluOpType.mult,
                        op1=mybir.AluOpType.mult,
                        accum_out=emp[:, ds(t, 1)],
                    )
                    e_tiles[t] = ep

                st = {"s": None}

                def chain_links(t):
                    ep = e_tiles[t]
                    for j in range(4):
                        k = 4 * t + j
                        if k == 0:
                            st["s"] = ep[:, 0:256]
                            continue
                        pr = prp.tile([128, 256], f32, name="pr_t",
                                      tag="pr_t")
                        for g in range(4):
                            nc.tensor.matmul(
                                pr[32 * g:32 * g + 32, :],
                                consts["bdfx"][32 * g:32 * g + 32,
                                               ds(32 * g, 32)],
                                st["s"][32 * g:32 * g + 32, :],
                                start=True, stop=True,
                                tile_position=(32 * g, 32 * g),
                            )
                        s_new = spool.tile([128, 256], bf16, name="s_t",
                                           tag="s_t")
                        nc.vector.tensor_mul(s_new[:], pr[:],
                                             ep[:, ds(j * 256, 256)])
                        st["s"] = s_new[:]

                produce_dma(0, split=2)
                if _rep == 0:
                    load_consts()
                produce_dma(1)
                for t in range(NT):
                    if t + 2 < NT:
                        produce_dma(t + 2)
                    produce_tau(t)
                    chain_links(t)

                # ---- final: beta_31 to partitions 0..63, pair, reduce ----
                s31 = st["s"]
                prf = pfp.tile([64, 256], f32)
                nc.tensor.matmul(prf[0:32, :], consts["wfin"][64:96, 0:32],
                                 s31[64:96, :], start=True, stop=True,
                                 tile_position=(64, 0))
                nc.tensor.matmul(prf[32:64, :], consts["wfin"][96:128, 32:64],
                                 s31[96:128, :], start=True, stop=True,
                                 tile_position=(96, 32))
                u = spool.tile([64, 256], bf16, name="u_t", tag="s_t")
                nc.vector.tensor_mul(u[:], prf[:], s31[0:64, :])
                pl = plp.tile([2, 256], f32)
                nc.tensor.matmul(pl[:], consts["ones2"][:], u[:],
                                 start=True, stop=True)
                nc.scalar.activation(lns[:], pl[:],
                                     mybir.ActivationFunctionType.Ln)

            nc.sync.dma_start(EMP_d[:], emp[:])
            nc.sync.dma_start(LNS_d[:], lns[:])

    nc.compile()
    _PROG[reps] = nc
    return nc


def host_prep(X, labels, W, T, K):
    """Build per-core device inputs + host-side scalars."""
    X = np.asarray(X, dtype=np.float32)
    labels = np.asarray(labels).astype(np.int64)
    W = np.asarray(W, dtype=np.float32)
    T = np.asarray(T, dtype=np.float32)
    K5 = np.asarray(K, dtype=np.float64).reshape(5, 5)

    C = _conv_matrix(K5)
    G = C @ W.astype(np.float64)                    # (D, Y)
    G32 = np.zeros((D, 32), dtype=_F8)
    G32[:, :Y] = (G * GS).astype(np.float32).astype(_F8)

    M = (np.exp(T.astype(np.float64)) / Y).astype(np.float32)
    BDFX = np.zeros((128, 128), dtype=_BF16)
    WFIN = np.zeros((128, 64), dtype=_BF16)
    for g in range(4):
        blk = M if g < 2 else M.T
        BDFX[32 * g:32 * g + Y, 32 * g:32 * g + Y] = blk.astype(_BF16)
    for g in range(2):
        # lhsT[64+32g+y, 32g+y'] = M[y', y]  ->  out = M^T gamma, shifted
        WFIN[64 + 32 * g:64 + 32 * g + Y, 32 * g:32 * g + Y] = \
            M.T.astype(_BF16)
    ONES = np.zeros((64, 2), dtype=_BF16)
    for g in range(2):
        ONES[32 * g:32 * g + Y, g] = 1.0

    Xq = X.astype(_F8)                              # (B, L, D)

    # index maps for the XT column layout:
    # col = tau*4096 + h*2048 + g*512 + jj*256 + c   (j = 2h + jj)
    tau = np.arange(NT)[:, None, None, None, None]
    h = np.arange(2)[None, :, None, None, None]
    g = np.arange(4)[None, None, :, None, None]
    jj = np.arange(2)[None, None, None, :, None]
    c = np.arange(256)[None, None, None, None, :]
    j = 2 * h + jj
    t_fwd = 4 * tau + j
    t_idx = np.where(g < 2, t_fwd, 63 - t_fwd)      # (NT,2,4,2,256)
    w_idx = (g % 2) * 256 + c + 0 * tau
    t_idx = np.broadcast_to(t_idx, (NT, 2, 4, 2, 256))
    w_idx = np.broadcast_to(w_idx, (NT, 2, 4, 2, 256))

    # one-hot columns: q = h*512 + jj*256 + c per tau, partition 32g+label
    part_g = 32 * g
    col_q = (tau * 1024 + h * 512 + jj * 256 + c)
    col_q = np.broadcast_to(col_q, (NT, 2, 4, 2, 256))

    in_maps = []
    for cidx in range(NCORES):
        Xc = Xq[cidx * WPC:(cidx + 1) * WPC]        # (512, 64, 128) fp8
        XT = np.ascontiguousarray(
            Xc[w_idx, t_idx].reshape(-1, D).T)      # (128, 32768)

        lab = labels[cidx * WPC:(cidx + 1) * WPC]   # (512, 64)
        prt = (part_g + lab[w_idx, t_idx])          # (NT,2,4,2,256)
        OHT = np.zeros((128, NT * 1024), dtype=np.float32)
        OHT[prt.ravel(), col_q.ravel()] = 1.0
        in_maps.append({
            "XT": XT,
            "OHT": OHT.astype(_BF16),
            "G32": G32,
            "BDFX": BDFX,
            "WFIN": WFIN,
            "ONES2": ONES,
        })

    tr_total = float(T.astype(np.float64)[labels[:, :-1], labels[:, 1:]].sum())
    reg = 0.5 * float(np.sum(W.astype(np.float64) ** 2)) \
        + 0.5 * float(np.sum(T.astype(np.float64) ** 2))
    return in_maps, tr_total, reg, G32


def host_finish(results, tr_total, reg):
    em_total = 0.0
    lz_raw = 0.0
    for c in range(NCORES):
        em_total += float(results[c]["EMP"].astype(np.float64).sum())
        lz_raw += float(results[c]["LNS"].astype(np.float64).sum())
    logZ_total = lz_raw + B * (L - 1) * np.log(float(Y))
    loglik_sum = em_total + tr_total - logZ_total
    f = -C_REG * loglik_sum / B + reg
    return np.float32(f)


def kernel(X, labels, W, T, K):
    from concourse.bass_utils import run_bass_kernel_spmd

    nc = _build_program()
    in_maps, tr_total, reg, _ = host_prep(X, labels, W, T, K)
    last_err = None
    for _attempt in range(3):
        try:
            res = run_bass_kernel_spmd(nc, in_maps, list(range(NCORES)))
            out = host_finish(res.results, tr_total, reg)
            if np.isfinite(out):
                return out
            last_err = RuntimeError(f"non-finite result {out}")
        except Exception as e:   # transient device errors: retry
            last_err = e
    raise last_err


# revision 13
# speedup vs baseline: 1.1407x; 1.1407x over previous
"""Trainium2 Bass kernel for the CRF problem (v3).

Math:
  feat = conv2d(X.view(-1,1,16,8), K, pad=2)  -> flatten      (B, L, D)
  e    = feat @ W                                              (B, L, Y)
Both are linear in X, so fold:  e = X @ G  with  G = C_K @ W  (D x Y),
C_K the 128x128 conv matrix built from the 5x5 kernel (host prep, tiny).

logZ via the *scaled* forward algorithm; fwd and bwd chains share ONE
stationary block-diagonal weight set (no per-link weight swaps):

  S tiles [128, 128] (two column-halves A, B run as independent chains
  in antiphase so the PE matmul latency hides under the other half's
  DVE multiply):
    partitions = 4 strips of 32 states;
    strip 0,1 = fwd alpha (words c, c+256), strip 2,3 = bwd gamma.
  BDFX = diag(M, M, M^T, M^T), M = exp(T)/Y. 31 links:
    S_k = E_k ⊙ (BDFX-blockdiag @ S_{k-1}),  E_k top = exp(e) at t=k,
    E_k bottom = exp(e) at t=63-k.  Init S_0 = E_0.
  final: beta_31 = M^T gamma_32 shifted to partitions 0..63 via WFIN;
  u = alpha_31 ⊙ beta_31; per word logZ = ln Σ_y u + 63 ln Y.

Engine assignment (keeps the latency-critical chain alone on DVE):
  PE : e-matmuls (fp8e3 inputs), chain diag matmuls, em reduction
       (accumulating ones-matmuls into one PSUM bank)
  ACT: exp (per half-bank for early chain start), raw-e copy to SBUF
  GPS: em mask-multiply (raw-e x one-hot) in SBUF
  DVE: chain multiplies only
"""

import numpy as np
import ml_dtypes

B, L, D, Y = 4096, 64, 128, 26
NCORES = 8
WPC = B // NCORES          # 512 words per core
NT = 8                     # tau blocks; tau covers links k=4t..4t+3
C_REG = 1000.0
GS = 256.0                 # G scale so fp8e3 stays in normal range
NWARM = 6                  # PE warm-up matmuls during the DMA fill

_BF16 = ml_dtypes.bfloat16
_F8 = ml_dtypes.float8_e3m4
_PROG = {}


def _conv_matrix(K5):
    """C[q, p]: flattened-input q contribution to flattened-output p."""
    H, Wd = 16, 8
    C = np.zeros((D, D), dtype=np.float64)
    for oh in range(H):
        for ow in range(Wd):
            p = oh * Wd + ow
            for kh in range(5):
                for kw in range(5):
                    ih, iw = oh + kh - 2, ow + kw - 2
                    if 0 <= ih < H and 0 <= iw < Wd:
                        C[ih * Wd + iw, p] = K5[kh, kw]
    return C


def _build_program(reps=1):
    if reps in _PROG:
        return _PROG[reps]
    import concourse.tile as tile
    import concourse.mybir as mybir
    from concourse import bacc
    from concourse.bass import ds, ts

    f32 = mybir.dt.float32
    bf16 = mybir.dt.bfloat16
    f8 = mybir.dt.float8e3

    nc = bacc.Bacc("TRN2", target_bir_lowering=False, debug=False,
                   num_devices=NCORES)

    XT_d = nc.dram_tensor("XT", [D, WPC * L], f8, kind="ExternalInput")
    OHT_d = nc.dram_tensor("OHT", [128, NT * 1024], bf16,
                           kind="ExternalInput")
    G32_d = nc.dram_tensor("G32", [D, 32], f8, kind="ExternalInput")
    BDFX_d = nc.dram_tensor("BDFX", [128, 128], bf16, kind="ExternalInput")
    WFIN_d = nc.dram_tensor("WFIN", [128, 64], bf16, kind="ExternalInput")
    ONES_d = nc.dram_tensor("ONES2", [64, 2], bf16, kind="ExternalInput")
    ONESH_d = nc.dram_tensor("ONESH", [128, 4], bf16, kind="ExternalInput")
    EMP_d = nc.dram_tensor("EMP", [2, 512], f32, kind="ExternalOutput")
    LNS_d = nc.dram_tensor("LNS", [2, 256], f32, kind="ExternalOutput")

    with tile.TileContext(nc) as tc:
        with (
            tc.tile_pool(name="const", bufs=1) as cpool,
            tc.tile_pool(name="xt", bufs=4) as xtp,
            tc.tile_pool(name="ep", bufs=3) as epool,
            tc.tile_pool(name="er", bufs=2) as erp,
            tc.tile_pool(name="scr", bufs=2) as scrp,
            tc.tile_pool(name="s", bufs=6) as spool,
            tc.tile_pool(name="out", bufs=1) as opool,
            tc.tile_pool(name="pe", bufs=2, space="PSUM") as pep,
            tc.tile_pool(name="pr", bufs=2, space="PSUM") as prp,
            tc.tile_pool(name="pm", bufs=1, space="PSUM") as pmp,
        ):
            consts = {}
            emp = opool.tile([2, 512], f32)
            lns = opool.tile([2, 256], f32)
            emp_ps = pmp.tile([2, 512], f32)

            # ---- PE warm-up: keep HAM busy while the first DMA lands ----
            warm = cpool.tile([128, 256], bf16)
            nc.vector.memset(warm[:], 0.0)
            pw = prp.tile([128, 128], f32, name="pw", tag="pr_t")
            for _ in range(NWARM):
                nc.tensor.matmul(pw[:], warm[:, 0:128], warm[:, 0:128],
                                 start=True, stop=True)

            def load_consts():
                # via the gpsimd (SWDGE) queue: keeps the SP HWDGE ring
                # free for the XT/OHT stream
                g32 = cpool.tile([D, 32], f8)
                nc.gpsimd.dma_start(g32[:], G32_d[:])
                bdfx = cpool.tile([128, 128], bf16)
                nc.gpsimd.dma_start(bdfx[:], BDFX_d[:])
                wfin = cpool.tile([128, 64], bf16)
                nc.gpsimd.dma_start(wfin[:], WFIN_d[:])
                ones2 = cpool.tile([64, 2], bf16)
                nc.gpsimd.dma_start(ones2[:], ONES_d[:])
                onesh = cpool.tile([128, 4], bf16)
                nc.gpsimd.dma_start(onesh[:], ONESH_d[:])
                oht = cpool.tile([128, NT * 1024], bf16)
                consts.update(g32=g32, bdfx=bdfx, wfin=wfin, ones2=ones2,
                              onesh=onesh, oht=oht)

            for _rep in range(reps):
                xt_tiles = {}
                e_tiles = {}
                er_tiles = {}
                emn = [0]

                def dma_xt(t, split=1):
                    xt = xtp.tile([D, 4096], f8, name="xt_t", tag="xt_t")
                    w = 4096 // split
                    for c in range(split):
                        nc.sync.dma_start(
                            xt[:, ds(c * w, w)],
                            XT_d[:, ds(t * 4096 + c * w, w)])
                    xt_tiles[t] = xt

                def dma_oht(t):
                    nc.sync.dma_start(consts["oht"][:, ts(t, 1024)],
                                      OHT_d[:, ts(t, 1024)])

                def produce_half(t, h):
                    # 4 col-strip matmuls -> pe half; exp to E tile half
                    xt = xt_tiles[t]
                    if h == 0:
                        pe = pep.tile([128, 1024], f32, name="pe_t",
                                      tag="pe_t")
                        e_tiles[t] = (
                            epool.tile([128, 1024], bf16, name="e_t",
                                       tag="e_t"),
                            pe)
                    ep, pe = e_tiles[t]
                    for g in range(4):
                        nc.tensor.matmul(
                            pe[32 * g:32 * g + 32, ds(h * 512, 512)],
                            consts["g32"][:],
                            xt[:, ds(h * 2048 + g * 512, 512)],
                            start=True, stop=True,
                            tile_position=(0, 32 * g),
                        )
                    nc.scalar.activation(ep[:, ds(h * 512, 512)],
                                         pe[:, ds(h * 512, 512)],
                                         mybir.ActivationFunctionType.Exp,
                                         scale=1.0 / GS)

                def produce_em(t):
                    # raw e to SBUF (ACT), mask-mul (GPSIMD), partition
                    # reduction via accumulating ones-matmuls (PE)
                    ep, pe = e_tiles[t]
                    er = erp.tile([128, 1024], bf16, name="er_t", tag="er_t")
                    nc.scalar.mul(er[:], pe[:], 1.0 / GS)
                    er_tiles[t] = er
                    scr = scrp.tile([128, 1024], bf16, name="sc_t",
                                    tag="sc_t")
                    nc.gpsimd.tensor_mul(scr[:], er[:],
                                         consts["oht"][:, ts(t, 1024)])
                    for h in range(2):
                        i = emn[0]
                        nc.tensor.matmul(
                            emp_ps[:], consts["onesh"][:, ds(2 * h, 2)],
                            scr[:, ds(h * 512, 512)],
                            start=(i == 0), stop=(i == 2 * NT - 1),
                            skip_group_check=True,
                        )
                        emn[0] += 1

                st = {"a": None, "b": None}

                def link(t, j, half):
                    k = 4 * t + j
                    ep, _pe = e_tiles[t]
                    esl = ep[:, ds(j * 256 + half * 128, 128)]
                    key = "a" if half == 0 else "b"
                    if k == 0:
                        st[key] = esl
                        return
                    pr = prp.tile([128, 128], f32, name="pr_t", tag="pr_t")
                    for g in range(4):
                        nc.tensor.matmul(
                            pr[32 * g:32 * g + 32, :],
                            consts["bdfx"][32 * g:32 * g + 32,
                                           ds(32 * g, 32)],
                            st[key][32 * g:32 * g + 32, :],
                            start=True, stop=True,
                            tile_position=(32 * g, 32 * g),
                        )
                    s_new = spool.tile([128, 128], bf16, name="s_t",
                                       tag="s_t")
                    nc.vector.tensor_mul(s_new[:], pr[:], esl)
                    st[key] = s_new[:]

                # ---- prologue: DMAs, consts, first tau ----
                dma_xt(0, split=2)
                if _rep == 0:
                    load_consts()
                dma_xt(1)
                dma_oht(0)
                dma_xt(2)
                produce_half(0, 0)
                produce_half(0, 1)
                dma_oht(1)

                for t in range(NT):
                    for j in range(4):
                        link(t, j, 0)
                        link(t, j, 1)
                        # interleave next-tau production at link grain
                        if t + 1 < NT:
                            if j < 2:
                                produce_half(t + 1, j)
                            elif j == 2:
                                produce_em(t)
                                if t + 3 < NT:
                                    dma_xt(t + 3)
                                if t + 2 < NT:
                                    dma_oht(t + 2)
                        elif j == 2:
                            produce_em(t)

                # ---- final: beta_31 to partitions 0..63, pair, reduce ----
                u = spool.tile([64, 256], bf16, name="u_t", tag="u_t")
                for half, key in ((0, "a"), (1, "b")):
                    s31 = st[key]
                    prf = prp.tile([64, 128], f32, name="pf_t", tag="pr_t")
                    nc.tensor.matmul(prf[0:32, :],
                                     consts["wfin"][64:96, 0:32],
                                     s31[64:96, :], start=True, stop=True,
                                     tile_position=(64, 0))
                    nc.tensor.matmul(prf[32:64, :],
                                     consts["wfin"][96:128, 32:64],
                                     s31[96:128, :], start=True, stop=True,
                                     tile_position=(96, 32))
                    nc.vector.tensor_mul(u[:, ds(half * 128, 128)],
                                         prf[:], s31[0:64, :])
                pl = prp.tile([2, 256], f32, name="pl_t", tag="pr_t")
                nc.tensor.matmul(pl[:], consts["ones2"][:], u[:],
                                 start=True, stop=True)
                nc.scalar.activation(lns[:], pl[:],
                                     mybir.ActivationFunctionType.Ln)
                nc.scalar.copy(emp[:], emp_ps[:])

            nc.sync.dma_start(EMP_d[:], emp[:])
            nc.sync.dma_start(LNS_d[:], lns[:])

    nc.compile()
    _PROG[reps] = nc
    return nc


def host_prep(X, labels, W, T, K):
    """Build per-core device inputs + host-side scalars."""
    X = np.asarray(X, dtype=np.float32)
    labels = np.asarray(labels).astype(np.int64)
    W = np.asarray(W, dtype=np.float32)
    T = np.asarray(T, dtype=np.float32)
    K5 = np.asarray(K, dtype=np.float64).reshape(5, 5)

    C = _conv_matrix(K5)
    G = C @ W.astype(np.float64)                    # (D, Y)
    G32 = np.zeros((D, 32), dtype=_F8)
    G32[:, :Y] = (G * GS).astype(np.float32).astype(_F8)

    M = (np.exp(T.astype(np.float64)) / Y).astype(np.float32)
    BDFX = np.zeros((128, 128), dtype=_BF16)
    WFIN = np.zeros((128, 64), dtype=_BF16)
    for g in range(4):
        blk = M if g < 2 else M.T
        BDFX[32 * g:32 * g + Y, 32 * g:32 * g + Y] = blk.astype(_BF16)
    for g in range(2):
        # lhsT[64+32g+y, 32g+y'] = M[y', y]  ->  out = M^T gamma, shifted
        WFIN[64 + 32 * g:64 + 32 * g + Y, 32 * g:32 * g + Y] = \
            M.T.astype(_BF16)
    ONES = np.zeros((64, 2), dtype=_BF16)
    for g in range(2):
        ONES[32 * g:32 * g + Y, g] = 1.0
    ONESH = np.zeros((128, 4), dtype=_BF16)
    ONESH[:, 0] = 1.0     # h=0 -> row 0
    ONESH[:, 3] = 1.0     # h=1 -> row 1

    Xq = X.astype(_F8)                              # (B, L, D)

    # index maps for the XT column layout:
    # col = tau*4096 + h*2048 + g*512 + jj*256 + c   (j = 2h + jj)
    tau = np.arange(NT)[:, None, None, None, None]
    h = np.arange(2)[None, :, None, None, None]
    g = np.arange(4)[None, None, :, None, None]
    jj = np.arange(2)[None, None, None, :, None]
    c = np.arange(256)[None, None, None, None, :]
    j = 2 * h + jj
    t_fwd = 4 * tau + j
    t_idx = np.where(g < 2, t_fwd, 63 - t_fwd)      # (NT,2,4,2,256)
    w_idx = (g % 2) * 256 + c + 0 * tau
    t_idx = np.broadcast_to(t_idx, (NT, 2, 4, 2, 256))
    w_idx = np.broadcast_to(w_idx, (NT, 2, 4, 2, 256))

    # one-hot columns: q = h*512 + jj*256 + c per tau, partition 32g+label
    part_g = 32 * g
    col_q = (tau * 1024 + h * 512 + jj * 256 + c)
    col_q = np.broadcast_to(col_q, (NT, 2, 4, 2, 256))

    in_maps = []
    for cidx in range(NCORES):
        Xc = Xq[cidx * WPC:(cidx + 1) * WPC]        # (512, 64, 128) fp8
        XT = np.ascontiguousarray(
            Xc[w_idx, t_idx].reshape(-1, D).T)      # (128, 32768)

        lab = labels[cidx * WPC:(cidx + 1) * WPC]   # (512, 64)
        prt = (part_g + lab[w_idx, t_idx])          # (NT,2,4,2,256)
        OHT = np.zeros((128, NT * 1024), dtype=np.float32)
        OHT[prt.ravel(), col_q.ravel()] = 1.0
        in_maps.append({
            "XT": XT,
            "OHT": OHT.astype(_BF16),
            "G32": G32,
            "BDFX": BDFX,
            "WFIN": WFIN,
            "ONES2": ONES,
            "ONESH": ONESH,
        })

    tr_total = float(T.astype(np.float64)[labels[:, :-1], labels[:, 1:]].sum())
    reg = 0.5 * float(np.sum(W.astype(np.float64) ** 2)) \
        + 0.5 * float(np.sum(T.astype(np.float64) ** 2))
    return in_maps, tr_total, reg, G32


def host_finish(results, tr_total, reg):
    em_total = 0.0
    lz_raw = 0.0
    for c in range(NCORES):
        em_total += float(results[c]["EMP"].astype(np.float64).sum())
        lz_raw += float(results[c]["LNS"].astype(np.float64).sum())
    logZ_total = lz_raw + B * (L - 1) * np.log(float(Y))
    loglik_sum = em_total + tr_total - logZ_total
    f = -C_REG * loglik_sum / B + reg
    return np.float32(f)


def kernel(X, labels, W, T, K):
    from concourse.bass_utils import run_bass_kernel_spmd

    nc = _build_program()
    in_maps, tr_total, reg, _ = host_prep(X, labels, W, T, K)
    last_err = None
    for _attempt in range(3):
        try:
            res = run_bass_kernel_spmd(nc, in_maps, list(range(NCORES)))
            out = host_finish(res.results, tr_total, reg)
            if np.isfinite(out):
                return out
            last_err = RuntimeError(f"non-finite result {out}")
        except Exception as e:   # transient device errors: retry
            last_err = e
    raise last_err
